# revision 3
# baseline (speedup 1.0000x reference)
"""Cubic B-spline FFD 3D upsampling kernel for Trainium2 (8 NeuronCores), v3.

v [4,3,44,52,44] f32 -> out [4,3,160,192,160] f32 via three separable stride-4
transposed convs (cubic B-spline, len 15) + crop. Sharded over output z: core c
reads input planes [5c,5c+8), writes zo [20c,20c+20).

Per-core pipeline (fp16 compute, f32 PSUM):
  L0 [128=(g@64, yi52), (b6, zi8, xi44)]        <- 1 DMA per g, 4224B runs
  z-pass on DVE: tensor_scalar_mul (4x mode) + tensor_add (2x), partitions
    [0:116] in one instr (rows 52:64 zeroed), split into two b-halves
  L1 [128, (b6, zo20, xp64)]  (x-pad cols zeroed once)
  y-pass on PE: zo-pair matmuls M=128: lhsT=L1[yi, (zo2,xp64)], rhs=Wy[52,192]
    -> py [128=(i_z@64, xi64), 192] f32, 2 matmuls/bank
  copy f32->f16 (DVE/ACT balanced) -> L2b [128, (g2, zp10, yo192)]
  x-pass on PE: lhsT=L2b[64*i_z+(0:44), m-chunk128], rhs=Wx[44,160]
    -> px [128=m-chunk, 160] f32, 6 matmuls per 2-bank tile
  copy f32->f16 -> st [128, (q10=(i_z,u), jx480)]  (p-major layout)
  DMA st -> dev out [12, 10, 128, 480] f16, 960B contiguous runs
Host: permutation fixup + f16->f32.
"""

import os
import numpy as np

N_CORES = 8
ZIN, YIN, XIN = 44, 52, 44
ZOUT, YOUT, XOUT = 160, 192, 160
BC = 12
ZSH = ZOUT // N_CORES      # 20
ZISH = 8
XP = 64


def _bspline_kernel():
    x = (np.arange(15) - 7) / 4.0
    t = np.abs(x)
    return np.where(
        t < 1.0, 2.0 / 3.0 + (0.5 * t - 1.0) * t**2,
        np.where(t < 2.0, ((2.0 - t) ** 3) / 6.0, 0.0)
    ).astype(np.float32)


_W = _bspline_kernel()


def _exp_mat(n_in, n_out):
    """M[i, o] = weight of control point i on (post-crop) output o."""
    M = np.zeros((n_in, n_out), dtype=np.float32)
    for o in range(n_out):
        for i in range(n_in):
            n = 4 * i - o + 3
            if 0 <= n < 15:
                M[i, o] = _W[n]
    return M


def _ztaps():
    """Per phase r: list of (tap t, weight); input plane = k + t for zo=4k+r."""
    out = []
    for r in range(4):
        taps = []
        for t in range(4):
            n = 4 * t + 3 - r
            if 0 <= n < 15:
                taps.append((t, float(_W[n])))
        out.append(taps)
    return out


_NC_CACHE = {}

# measured marginal per-instruction costs (ns) for copy load balancing
_DVE_X, _DVE_Y = 1125.0, 570.0
_ACT_X, _ACT_Y = 1045.0, 520.0
_Z_DVE_NS = 8000.0


def _build_nc():
    import concourse.bacc as bacc
    import concourse.mybir as mybir
    from concourse.tile import TileContext

    FP32 = mybir.dt.float32
    FP16 = mybir.dt.float16

    nc = bacc.Bacc()
    v = nc.declare_dram_parameter("v", [2, YIN, 6 * ZISH * XIN], FP16,
                                  isOutput=False)
    wy = nc.declare_dram_parameter("wy", [128, YOUT], FP16, isOutput=False)
    wx = nc.declare_dram_parameter("wx", [128, XOUT], FP16, isOutput=False)
    out = nc.declare_dram_parameter("out", [BC, 10, 128, 480], FP16,
                                    isOutput=True)

    ztaps = _ztaps()
    eng_busy = {"dve": _Z_DVE_NS, "act": 0.0}
    dma_ctr = [0]
    zctr = [0]

    alt = [0]

    def pick_copy_engine(dve_cost, act_cost):
        if os.environ.get("STRICT_ALT"):
            alt[0] += 1
            return "dve" if alt[0] % 2 == 0 else "act"
        if eng_busy["dve"] + dve_cost <= eng_busy["act"] + act_cost:
            eng_busy["dve"] += dve_cost
            return "dve"
        eng_busy["act"] += act_cost
        return "act"

    with TileContext(nc) as tc:
        with (
            tc.tile_pool(name="const", bufs=1) as cpool,
            tc.tile_pool(name="io", bufs=1) as iopool,
            tc.tile_pool(name="l2", bufs=2) as l2pool,
            tc.tile_pool(name="st", bufs=4) as stpool,
            tc.tile_pool(name="psy", bufs=2, space="PSUM") as psy,
            tc.tile_pool(name="psx", bufs=3, space="PSUM") as psx,
        ):
            wyt = cpool.tile([128, YOUT], FP16)
            nc.sync.dma_start(out=wyt[:, :], in_=wy[:, :])
            wxt = cpool.tile([128, XOUT], FP16)
            nc.sync.dma_start(out=wxt[:, :], in_=wx[:, :])

            warm = cpool.tile([128, 512], FP16)
            nc.vector.memset(warm[:, :], 0.0)
            pw = psy.tile([128, 512], FP32, name="py")
            for w in range(24):
                nc.tensor.matmul(pw[:, 0:512], lhsT=warm[0:52, 0:128],
                                 rhs=warm[0:52, :], start=True, stop=True,
                                 skip_group_check=True)

            L0 = iopool.tile([128, 6 * ZISH * XIN], FP16)
            nc.gpsimd.memset(L0[32:64, :], 0.0)
            for g in range(2):
                nc.sync.dma_start(out=L0[64 * g:64 * g + YIN, :], in_=v[g])

            # three L1 chunks (b 0 / 1-2 / 3-5) so y-pass starts early
            ZB = [(0, 2), (2, 6)]
            L1s = []
            for h in range(2):
                nb = ZB[h][1] - ZB[h][0]
                L1h = iopool.tile([128, nb * ZSH * XP], FP16, name=f"L1_{h}")
                L1s.append(L1h)
            tmp = iopool.tile([128, 6 * 5 * XIN], FP16)

            L0v = L0.rearrange("p (b z x) -> p b z x", b=6, z=ZISH)

            # ---- z-pass on DVE: muls (4x) + adds (2x), partitions [0:116]
            for h in range(2):
                b0, b1 = ZB[h]
                nb = b1 - b0
                L1r = L1s[h].rearrange("p (b k r x) -> p b k r x",
                                       b=nb, k=5, r=4)
                tvh = tmp.rearrange("p (b k x) -> p b k x", b=6, k=5)
                for r in range(4):
                    if os.environ.get("SKIP_Z"):
                        break
                    dst = L1r[0:116, :, :, r, 0:XIN]
                    t0, w0 = ztaps[r][0]
                    nc.vector.tensor_scalar_mul(
                        dst, L0v[0:116, b0:b1, t0:t0 + 5, :], w0)
                    for t, w in ztaps[r][1:]:
                        nc.vector.tensor_scalar_mul(
                            tvh[0:116, 0:nb, :, :],
                            L0v[0:116, b0:b1, t:t + 5, :], w)
                        nc.vector.tensor_add(dst, dst,
                                             tvh[0:116, 0:nb, :, :])

            outv = out  # [12, 10, 128, 480]

            def emit_y(b):
                """y-pass tile generator for batch-slot b (both g)."""
                L2b = l2pool.tile([128, 3840], FP16)
                h = 0 if b < ZB[0][1] else 1
                nbh = ZB[h][1] - ZB[h][0]
                L1q = L1s[h].rearrange("p (b zp q) -> p b zp q",
                                       b=nbh, zp=10)
                bb = b - ZB[h][0]

                def tile(t2):
                    py = psy.tile([128, 512], FP32)
                    for s in range(2):
                        p2 = 2 * t2 + s
                        g, zp = p2 // 10, p2 % 10
                        lo = 64 * g
                        nc.tensor.matmul(
                            py[:, 192 * s:192 * s + 192],
                            lhsT=L1q[lo:lo + YIN, bb, zp, :],
                            rhs=wyt[lo:lo + YIN, :], start=True, stop=True)
                    dst = L2b[:, 384 * t2:384 * t2 + 384]
                    if pick_copy_engine(_DVE_Y, _ACT_Y) == "dve":
                        nc.vector.tensor_copy(out=dst, in_=py[:, 0:384])
                    else:
                        nc.scalar.copy(dst, py[:, 0:384])
                return L2b, [lambda t2=t2: tile(t2) for t2 in range(10)]

            def emit_x_tile(b, g, bp, stt, q6, L2b):
                """One x psum tile (6 chunks) + copy (+DMA per 2 q-slots)."""
                L2f = L2b
                px = psx.tile([128, 1024], FP32)
                for s in range(6):
                    cg = 6 * q6 + s
                    iz, c = cg // 15, cg % 15
                    lo = 64 * iz
                    col = (s // 3) * 512 + (s % 3) * 160
                    nc.tensor.matmul(
                        px[:, col:col + 160],
                        lhsT=L2f[lo:lo + XIN,
                                 g * 1920 + 128 * c:
                                 g * 1920 + 128 * (c + 1)],
                        rhs=wxt[lo:lo + XIN, :],
                        start=True, stop=True)
                src = px.rearrange("p (c q) -> p c q", c=2)[:, :, 0:480]
                dst = stt[:, 960 * q6:960 * q6 + 960]
                if pick_copy_engine(_DVE_X, _ACT_X) == "dve":
                    nc.vector.tensor_copy(out=dst, in_=src)
                else:
                    nc.scalar.copy(dst, src)
                if not os.environ.get("SKIP_ODMA"):
                    dma_ctr[0] += 1
                    eng = nc.sync if dma_ctr[0] % 2 == 0 else nc.gpsimd
                    dstd = outv[bp, 2 * q6:2 * q6 + 2]
                    eng.dma_start(
                        out=dstd.rearrange("q p f -> p q f"),
                        in_=stt[:, 960 * q6:960 * q6 + 960]
                        .rearrange("p (q f) -> p q f", q=2))

            def x_thunks(b, L2b):
                out = []
                for g in range(2):
                    bp = 6 * g + b
                    stt = stpool.tile([128, 10 * 480], FP16)
                    for q6 in range(5):
                        out.append(
                            lambda b=b, g=g, bp=bp, stt=stt, q6=q6:
                            emit_x_tile(b, g, bp, stt, q6, L2b))
                return out

            prev = None
            for b in range(6):
                L2b, ythunks = emit_y(b)
                xthunks = x_thunks(b - 1, prev) if prev is not None else []
                # interleave: y, x, x, y, x, x, ... (10 y + 10 x)
                yi = xi2 = 0
                order = []
                while yi < len(ythunks) or xi2 < len(xthunks):
                    if yi < len(ythunks):
                        order.append(ythunks[yi]); yi += 1
                    if xi2 < len(xthunks):
                        order.append(xthunks[xi2]); xi2 += 1
                for t in order:
                    t()
                prev = L2b
            for t in x_thunks(5, prev):
                t()

    nc.compile()
    return nc


def _get_nc():
    if "nc" not in _NC_CACHE:
        _NC_CACHE["nc"] = _build_nc()
    return _NC_CACHE["nc"]


def _prep_inputs(v):
    """Full v [4,3,44,52,44] f32 -> per-core input maps."""
    f16 = np.float16
    v = np.asarray(v).astype(np.float32).reshape(BC, ZIN, YIN, XIN)

    wy128 = np.zeros((128, YOUT), dtype=np.float32)
    wy128[0:YIN] = _exp_mat(YIN, YOUT)
    wy128[64:64 + YIN] = wy128[0:YIN]
    wx128 = np.zeros((128, XOUT), dtype=np.float32)
    wx128[0:XIN] = _exp_mat(XIN, XOUT)
    wx128[64:64 + XIN] = wx128[0:XIN]
    wy_h = wy128.astype(f16)
    wx_h = wx128.astype(f16)

    in_maps = []
    for c in range(N_CORES):
        slab = v[:, 5 * c:5 * c + ZISH]                    # [12, 8, 52, 44]
        slab = slab.reshape(2, 6, ZISH, YIN, XIN).transpose(0, 3, 1, 2, 4)
        slab = np.ascontiguousarray(slab).reshape(2, YIN, 6 * ZISH * XIN)
        in_maps.append({"v": slab.astype(f16), "wy": wy_h, "wx": wx_h})
    return in_maps


def _assemble(results):
    """Per-core dev outputs [12, 10, 128, 480] f16 -> full f32 output."""
    out = np.empty((BC, ZOUT, YOUT, XOUT), dtype=np.float32)
    for c in range(N_CORES):
        dev = np.asarray(results[c]["out"])              # [12,10,128,480]
        dev = dev.reshape(BC, 2, 5, 128, 3, XOUT)
        dev = dev.transpose(0, 1, 2, 4, 3, 5)            # [12,2,5,3,128,160]
        dev = dev.reshape(BC, 2, 10, 192, XOUT)          # m -> (zp, yo)
        dev = dev.transpose(0, 2, 1, 3, 4)               # [12,10,2,192,160]
        blk = dev.reshape(BC, ZSH, YOUT, XOUT)
        out[:, ZSH * c:ZSH * (c + 1)] = blk.astype(np.float32)
    return out.reshape(4, 3, ZOUT, YOUT, XOUT)


def kernel(v):
    from concourse.bass_utils import run_bass_kernel_spmd

    in_maps = _prep_inputs(v)
    nc = _get_nc()
    res = run_bass_kernel_spmd(nc, in_maps, core_ids=list(range(N_CORES)))
    return _assemble(res.results)


# revision 4
# speedup vs baseline: 1.0396x; 1.0396x over previous
"""Cubic B-spline FFD 3D upsampling kernel for Trainium2 (8 NeuronCores), v3.

v [4,3,44,52,44] f32 -> out [4,3,160,192,160] f32 via three separable stride-4
transposed convs (cubic B-spline, len 15) + crop. Sharded over output z: core c
reads input planes [5c,5c+8), writes zo [20c,20c+20).

Per-core pipeline (fp16 compute, f32 PSUM):
  L0 [128=(g@64, yi52), (b6, zi8, xi44)]        <- 1 DMA per g, 4224B runs
  z-pass on DVE: tensor_scalar_mul (4x mode) + tensor_add (2x), partitions
    [0:116] in one instr (rows 52:64 zeroed), split into two b-halves
  L1 [128, (b6, zo20, xp64)]  (x-pad cols zeroed once)
  y-pass on PE: zo-pair matmuls M=128: lhsT=L1[yi, (zo2,xp64)], rhs=Wy[52,192]
    -> py [128=(i_z@64, xi64), 192] f32, 2 matmuls/bank
  copy f32->f16 (DVE/ACT balanced) -> L2b [128, (g2, zp10, yo192)]
  x-pass on PE: lhsT=L2b[64*i_z+(0:44), m-chunk128], rhs=Wx[44,160]
    -> px [128=m-chunk, 160] f32, 6 matmuls per 2-bank tile
  copy f32->f16 -> st [128, (q10=(i_z,u), jx480)]  (p-major layout)
  DMA st -> dev out [12, 10, 128, 480] f16, 960B contiguous runs
Host: permutation fixup + f16->f32.
"""

import os
import numpy as np

N_CORES = 8
ZIN, YIN, XIN = 44, 52, 44
ZOUT, YOUT, XOUT = 160, 192, 160
BC = 12
ZSH = ZOUT // N_CORES      # 20
ZISH = 8
XP = 64


def _bspline_kernel():
    x = (np.arange(15) - 7) / 4.0
    t = np.abs(x)
    return np.where(
        t < 1.0, 2.0 / 3.0 + (0.5 * t - 1.0) * t**2,
        np.where(t < 2.0, ((2.0 - t) ** 3) / 6.0, 0.0)
    ).astype(np.float32)


_W = _bspline_kernel()


def _exp_mat(n_in, n_out):
    """M[i, o] = weight of control point i on (post-crop) output o."""
    M = np.zeros((n_in, n_out), dtype=np.float32)
    for o in range(n_out):
        for i in range(n_in):
            n = 4 * i - o + 3
            if 0 <= n < 15:
                M[i, o] = _W[n]
    return M


def _ztaps():
    """Per phase r: list of (tap t, weight); input plane = k + t for zo=4k+r."""
    out = []
    for r in range(4):
        taps = []
        for t in range(4):
            n = 4 * t + 3 - r
            if 0 <= n < 15:
                taps.append((t, float(_W[n])))
        out.append(taps)
    return out


_NC_CACHE = {}

# measured marginal per-instruction costs (ns) for copy load balancing
_DVE_X, _DVE_Y = 1158.8, 587.1
_ACT_X, _ACT_Y = 1045.0, 520.0
_Z_DVE_NS = 9000.0


def _build_nc():
    import concourse.bacc as bacc
    import concourse.mybir as mybir
    from concourse.tile import TileContext

    FP32 = mybir.dt.float32
    FP16 = mybir.dt.float16

    nc = bacc.Bacc()
    v = nc.declare_dram_parameter("v", [2, YIN, 6 * ZISH * XIN], FP16,
                                  isOutput=False)
    wy = nc.declare_dram_parameter("wy", [128, YOUT], FP16, isOutput=False)
    wx = nc.declare_dram_parameter("wx", [128, XOUT], FP16, isOutput=False)
    out = nc.declare_dram_parameter("out", [BC, 10, 128, 480], FP16,
                                    isOutput=True)

    ztaps = _ztaps()
    eng_busy = {"dve": _Z_DVE_NS, "act": 0.0}
    dma_ctr = [0]
    zctr = [0]

    alt = [0]

    def pick_copy_engine(dve_cost, act_cost):
        if os.environ.get("STRICT_ALT"):
            alt[0] += 1
            return "dve" if alt[0] % 2 == 0 else "act"
        if eng_busy["dve"] + dve_cost <= eng_busy["act"] + act_cost:
            eng_busy["dve"] += dve_cost
            return "dve"
        eng_busy["act"] += act_cost
        return "act"

    with TileContext(nc) as tc:
        with (
            tc.tile_pool(name="const", bufs=1) as cpool,
            tc.tile_pool(name="io", bufs=1) as iopool,
            tc.tile_pool(name="l2", bufs=2) as l2pool,
            tc.tile_pool(name="st", bufs=4) as stpool,
            tc.tile_pool(name="psy", bufs=2, space="PSUM") as psy,
            tc.tile_pool(name="psx", bufs=3, space="PSUM") as psx,
        ):
            wyt = cpool.tile([128, YOUT], FP16)
            nc.sync.dma_start(out=wyt[:, :], in_=wy[:, :])
            wxt = cpool.tile([128, XOUT], FP16)
            nc.sync.dma_start(out=wxt[:, :], in_=wx[:, :])

            warm = cpool.tile([128, 512], FP16)
            nc.vector.memset(warm[:, :], 0.0)
            pw = psy.tile([128, 512], FP32, name="py")
            for w in range(24):
                nc.tensor.matmul(pw[:, 0:512], lhsT=warm[0:52, 0:128],
                                 rhs=warm[0:52, :], start=True, stop=True,
                                 skip_group_check=True)

            L0 = iopool.tile([128, 6 * ZISH * XIN], FP16)
            nc.gpsimd.memset(L0[32:64, :], 0.0)
            for g in range(2):
                nc.sync.dma_start(out=L0[64 * g:64 * g + YIN, :], in_=v[g])

            # three L1 chunks (b 0 / 1-2 / 3-5) so y-pass starts early
            ZB = [(0, 2), (2, 6)]
            L1s = []
            for h in range(2):
                nb = ZB[h][1] - ZB[h][0]
                L1h = iopool.tile([128, nb * ZSH * XP], FP16, name=f"L1_{h}")
                L1s.append(L1h)
            tmp = iopool.tile([128, 6 * 5 * XIN], FP16)

            L0v = L0.rearrange("p (b z x) -> p b z x", b=6, z=ZISH)

            # ---- z-pass on DVE: muls (4x) + adds (2x), partitions [0:116]
            for h in range(2):
                b0, b1 = ZB[h]
                nb = b1 - b0
                L1r = L1s[h].rearrange("p (b k r x) -> p b k r x",
                                       b=nb, k=5, r=4)
                tvh = tmp.rearrange("p (b k x) -> p b k x", b=6, k=5)
                for r in range(4):
                    if os.environ.get("SKIP_Z"):
                        break
                    dst = L1r[0:116, :, :, r, 0:XIN]
                    t0, w0 = ztaps[r][0]
                    nc.vector.tensor_scalar_mul(
                        dst, L0v[0:116, b0:b1, t0:t0 + 5, :], w0)
                    for t, w in ztaps[r][1:]:
                        nc.vector.tensor_scalar_mul(
                            tvh[0:116, 0:nb, :, :],
                            L0v[0:116, b0:b1, t:t + 5, :], w)
                        nc.vector.tensor_add(dst, dst,
                                             tvh[0:116, 0:nb, :, :])

            outv = out  # [12, 10, 128, 480]

            def emit_y(b):
                """y-pass tile generator for batch-slot b (both g)."""
                L2b = l2pool.tile([128, 3840], FP16)
                h = 0 if b < ZB[0][1] else 1
                nbh = ZB[h][1] - ZB[h][0]
                L1q = L1s[h].rearrange("p (b zp q) -> p b zp q",
                                       b=nbh, zp=10)
                bb = b - ZB[h][0]

                def tile(t2):
                    py = psy.tile([128, 512], FP32)
                    for s in range(2):
                        p2 = 2 * t2 + s
                        g, zp = p2 // 10, p2 % 10
                        lo = 64 * g
                        nc.tensor.matmul(
                            py[:, 192 * s:192 * s + 192],
                            lhsT=L1q[lo:lo + YIN, bb, zp, :],
                            rhs=wyt[lo:lo + YIN, :], start=True, stop=True)
                    dst = L2b[:, 384 * t2:384 * t2 + 384]
                    if pick_copy_engine(_DVE_Y, _ACT_Y) == "dve":
                        nc.vector.tensor_copy(out=dst, in_=py[:, 0:384])
                    else:
                        nc.scalar.copy(dst, py[:, 0:384])
                return L2b, [lambda t2=t2: tile(t2) for t2 in range(10)]

            def emit_x_tile(b, g, bp, stt, q6, L2b):
                """One x psum tile (6 chunks) + copy (+DMA per 2 q-slots)."""
                L2f = L2b
                px = psx.tile([128, 1024], FP32)
                for s in range(6):
                    cg = 6 * q6 + s
                    iz, c = cg // 15, cg % 15
                    lo = 64 * iz
                    col = (s // 3) * 512 + (s % 3) * 160
                    nc.tensor.matmul(
                        px[:, col:col + 160],
                        lhsT=L2f[lo:lo + XIN,
                                 g * 1920 + 128 * c:
                                 g * 1920 + 128 * (c + 1)],
                        rhs=wxt[lo:lo + XIN, :],
                        start=True, stop=True)
                src = px.rearrange("p (c q) -> p c q", c=2)[:, :, 0:480]
                dst = stt[:, 960 * q6:960 * q6 + 960]
                if pick_copy_engine(_DVE_X, _ACT_X) == "dve":
                    nc.vector.tensor_copy(out=dst, in_=src)
                else:
                    nc.scalar.copy(dst, src)
                if not os.environ.get("SKIP_ODMA"):
                    dma_ctr[0] += 1
                    eng = nc.sync if dma_ctr[0] % 2 == 0 else nc.gpsimd
                    dstd = outv[bp, 2 * q6:2 * q6 + 2]
                    eng.dma_start(
                        out=dstd.rearrange("q p f -> p q f"),
                        in_=stt[:, 960 * q6:960 * q6 + 960]
                        .rearrange("p (q f) -> p q f", q=2))

            def x_thunks(b, L2b):
                out = []
                for g in range(2):
                    bp = 6 * g + b
                    stt = stpool.tile([128, 10 * 480], FP16)
                    for q6 in range(5):
                        out.append(
                            lambda b=b, g=g, bp=bp, stt=stt, q6=q6:
                            emit_x_tile(b, g, bp, stt, q6, L2b))
                return out

            prev = None
            for b in range(6):
                L2b, ythunks = emit_y(b)
                xthunks = x_thunks(b - 1, prev) if prev is not None else []
                # interleave: y, x, x, y, x, x, ... (10 y + 10 x)
                yi = xi2 = 0
                order = []
                while yi < len(ythunks) or xi2 < len(xthunks):
                    if yi < len(ythunks):
                        order.append(ythunks[yi]); yi += 1
                    if xi2 < len(xthunks):
                        order.append(xthunks[xi2]); xi2 += 1
                for t in order:
                    t()
                prev = L2b
            for t in x_thunks(5, prev):
                t()

    nc.compile()
    return nc


def _get_nc():
    if "nc" not in _NC_CACHE:
        _NC_CACHE["nc"] = _build_nc()
    return _NC_CACHE["nc"]


def _prep_inputs(v):
    """Full v [4,3,44,52,44] f32 -> per-core input maps."""
    f16 = np.float16
    v = np.asarray(v).astype(np.float32).reshape(BC, ZIN, YIN, XIN)

    wy128 = np.zeros((128, YOUT), dtype=np.float32)
    wy128[0:YIN] = _exp_mat(YIN, YOUT)
    wy128[64:64 + YIN] = wy128[0:YIN]
    wx128 = np.zeros((128, XOUT), dtype=np.float32)
    wx128[0:XIN] = _exp_mat(XIN, XOUT)
    wx128[64:64 + XIN] = wx128[0:XIN]
    wy_h = wy128.astype(f16)
    wx_h = wx128.astype(f16)

    in_maps = []
    for c in range(N_CORES):
        slab = v[:, 5 * c:5 * c + ZISH]                    # [12, 8, 52, 44]
        slab = slab.reshape(2, 6, ZISH, YIN, XIN).transpose(0, 3, 1, 2, 4)
        slab = np.ascontiguousarray(slab).reshape(2, YIN, 6 * ZISH * XIN)
        in_maps.append({"v": slab.astype(f16), "wy": wy_h, "wx": wx_h})
    return in_maps


def _assemble(results):
    """Per-core dev outputs [12, 10, 128, 480] f16 -> full f32 output."""
    out = np.empty((BC, ZOUT, YOUT, XOUT), dtype=np.float32)
    for c in range(N_CORES):
        dev = np.asarray(results[c]["out"])              # [12,10,128,480]
        dev = dev.reshape(BC, 2, 5, 128, 3, XOUT)
        dev = dev.transpose(0, 1, 2, 4, 3, 5)            # [12,2,5,3,128,160]
        dev = dev.reshape(BC, 2, 10, 192, XOUT)          # m -> (zp, yo)
        dev = dev.transpose(0, 2, 1, 3, 4)               # [12,10,2,192,160]
        blk = dev.reshape(BC, ZSH, YOUT, XOUT)
        out[:, ZSH * c:ZSH * (c + 1)] = blk.astype(np.float32)
    return out.reshape(4, 3, ZOUT, YOUT, XOUT)


def kernel(v):
    from concourse.bass_utils import run_bass_kernel_spmd

    in_maps = _prep_inputs(v)
    nc = _get_nc()
    res = run_bass_kernel_spmd(nc, in_maps, core_ids=list(range(N_CORES)))
    return _assemble(res.results)
